# revision 44
# baseline (speedup 1.0000x reference)
"""AttentionWithBias (AlphaFold-style gated attention with pair bias) on 8 trn2 cores.

v3: mask compaction + full-width weights. mask==0 kills whole key columns
(p=exp(-1e9-..)==0 exactly in f32) and whole query rows (output zeroed), so
kernel() compacts both host-side: each core gets the unmasked keys of its batch
(padded to mult 32, uniform across cores -> kpad) and a 1/4 share of the
unmasked queries. Queries are padded to 128 because PE weight loads are only
fast at exactly 128 (or <=32) columns: measured 30.6ns per (LDW+9-col MM) pair
at 128 cols vs 60.7ns at 72 cols and 80.7ns at 96. Since any mask gives
<=ceil(512/4)=128 queries per core, qpad=128 is also fully general.

All bias DMAs are issued up front (the compacted bias fits in SBUF whole);
issue order interleaves the first chunk's quarters ahead of the small tensors
so the PE starts projecting ~8us in, and x_k is pre-arranged host-side into
[128, nkr*256] so its DMA is one contiguous run per partition (a strided
rearrange DMA was observed to take 17us and stall the sync queue's 8-deep
DMA semaphore rotation).

Per 32-key quarter: square the quarter (engine chosen per quarter: DVE mostly,
one on ACT, late ones on GPSIMD to balance the drain), then per key a PE pair
{proj: lhsT=[128d, 128q] bias block, rhs=wext[:, 0:9] -> 8 pre-centered heads +
mean; sumsq: lhsT=squared block, rhs=ones}, pitch 10 in PSUM; per-quarter
rinv=exp(-.5 ln(sumsq/128 - mean^2 + eps)); t1 = piece*rinv releases the PSUM
piece. Phase0 (kT/qT/v_ext/gate/S logits) sits after chunk-0 projections in PE
program order. Fixup: t2 = t1 + S on DVE (strided head cols), p = exp(t2) on
ACT, PE transposes p per head, PV accumulates into one persistent PSUM bank
with an appended ones column giving softmax denominators. Zero bias rows
(ln_in_b=bo=0 here) elide their matmuls; the uniform gate bias bg folds into
the sigmoid's activation bias operand. Padded keys are masked via mk=-2e9;
padded query rows produce garbage that stays partition-contained and is never
DMA'd out (out is [qpad_real rows] only via host scatter).
"""

import sys

if "/opt/trn_rl_repo" not in sys.path:
    sys.path.insert(0, "/opt/trn_rl_repo")

from contextlib import ExitStack

import ml_dtypes
import numpy as np

import concourse.bacc as bacc
import concourse.bass as bass
import concourse.tile as tile
from concourse import masks, mybir
from concourse.bass_utils import run_bass_kernel_spmd

BF16 = ml_dtypes.bfloat16
F32 = mybir.dt.float32
BF = mybir.dt.bfloat16
AF = mybir.ActivationFunctionType
OP = mybir.AluOpType

D_IN = 256
D_BIAS = 128
H = 8
DH = 32
B = 2
L = 512
SCALE = 1.0 / np.sqrt(DH)
NEG = -2.0e9
EPS = 1e-5
P = 10            # PSUM column pitch per key: 8 heads + mean + sumsq
QP = 128          # query pad: full-width weights; always >= ceil(L/4)

_CACHE = {}


def _ap(base, off, dims):
    return bass.AP(tensor=base.tensor, offset=base.offset + off, ap=[list(base.ap[0])] + dims)


def _build(kpad, qsq, modes):
    """modes: per-projection bias handling for (q, k, v, g, o):
    0 = no bias row (all zero), 1 = uniform scalar (gate only; via act bias), 2 = matmul.
    qsq: number of real (padded-to-8) query columns; squares are computed only there."""
    nkr = (kpad + 127) // 128
    kfull = nkr * 128
    chunks = []
    rem = kpad
    while rem > 0:
        chunks.append(min(128, rem))
        rem -= 128
    NCH = len(chunks)

    nc = bacc.Bacc("TRN2", target_bir_lowering=False, debug=False, num_devices=8)
    need_ones_row = any(m == 2 for m in modes)

    bias_tr = nc.declare_dram_parameter("bias_tr", [D_BIAS, kpad, QP], BF, isOutput=False)
    # x_q | x_k | mk | rowm | gbias packed in ONE f32 tensor: the global DMA
    # semaphore ring is only ~10 deep and skinny/strided DMAs complete late,
    # so everything small must ride one wide contiguous DMA
    XC = D_IN + nkr * D_IN + kpad + 8
    xcomb = nc.declare_dram_parameter("xcomb", [128, XC], F32, isOutput=False)
    # wall = 5 projection weights [5,2,256] + wext (bias head proj) in cols 2560:2576
    wall = nc.declare_dram_parameter("wall", [128, 5 * 2 * D_IN + 16], BF, isOutput=False)
    if need_ones_row:
        brows = nc.declare_dram_parameter("brows", [5, D_IN], BF, isOutput=False)

    out = nc.declare_dram_parameter("out", [QP, D_IN], F32, isOutput=True)

    # engine for each quarter's square: DVE (2 el/cyc) except the last chunk's,
    # which go to ACT (free in the drain window). GPSIMD squares measured
    # ~4.6us/quarter - too slow for any critical window.
    def sq_engine(ci, g):
        return "act" if ci == NCH - 1 else "dve"

    with tile.TileContext(nc) as tc, ExitStack() as ctx:
        sing = ctx.enter_context(tc.tile_pool(name="sing", bufs=1))
        sqp = ctx.enter_context(tc.tile_pool(name="sqp", bufs=5))
        scr = ctx.enter_context(tc.tile_pool(name="scr", bufs=2))
        ptp = ctx.enter_context(tc.tile_pool(name="ptp", bufs=2))
        lnp = ctx.enter_context(tc.tile_pool(name="lnp", bufs=5))
        ps_raw = ctx.enter_context(tc.tile_pool(name="ps_raw", bufs=5, space="PSUM"))
        ps_pt = ctx.enter_context(tc.tile_pool(name="ps_pt", bufs=1, space="PSUM"))
        ps_pv = ctx.enter_context(tc.tile_pool(name="ps_pv", bufs=1, space="PSUM"))

        def ps_tile():
            return ps_raw.tile([128, 512], F32, tag="rawps", name="rawps")

        # ---------------- DMAs ----------------
        # The DMA semaphore ring is GLOBAL across queues and the scheduler
        # orders ring slots itself, so the total DMA count must fit the ring
        # (~10): wall, 4-5 chunk-0 pieces, xcomb, 2-3 later-chunk pieces, out.
        # Every DMA is contiguous >=1KB per partition so completions are prompt.
        wall_sb = sing.tile([128, 5 * 2 * D_IN + 16], BF)
        nc.sync.dma_start(out=wall_sb[:], in_=wall[:, :])
        w_sb = {n: wall_sb[:, i * 2 * D_IN:(i + 1) * 2 * D_IN].rearrange(
            "p (h c) -> p h c", h=2) for i, n in enumerate("qkvgo")}
        wext_sb = wall_sb[:, 5 * 2 * D_IN:5 * 2 * D_IN + 16]
        tbs = {}
        c0q = chunks[0] // 32
        for g in range(c0q):
            tbg = sing.tile([128, 32, QP], BF, name=f"tb0_{g}")
            if g == 0:  # first 16 keys land ~1.4us earlier; subtile deps cover it
                nc.sync.dma_start(out=tbg[:, 0:16, :], in_=bias_tr[:, 0:16, :])
                nc.sync.dma_start(out=tbg[:, 16:32, :], in_=bias_tr[:, 16:32, :])
            else:
                nc.sync.dma_start(out=tbg[:], in_=bias_tr[:, g * 32:(g + 1) * 32, :])
            tbs[(0, g)] = tbg
        xcomb_sb = sing.tile([128, XC], F32)
        nc.gpsimd.dma_start(out=xcomb_sb[:], in_=xcomb[:, :])
        xqt = xcomb_sb[:, 0:D_IN]
        xall = xcomb_sb[:, D_IN:D_IN + nkr * D_IN].rearrange("p (r c) -> p r c", r=nkr)
        mk_sb = xcomb_sb[:, (1 + nkr) * D_IN:(1 + nkr) * D_IN + kpad]
        rowm_sb = xcomb_sb[:, (1 + nkr) * D_IN + kpad:(1 + nkr) * D_IN + kpad + 1]
        gb_sb = xcomb_sb[:, (1 + nkr) * D_IN + kpad + 1:(1 + nkr) * D_IN + kpad + 2]
        if need_ones_row:
            brow_sb = sing.tile([1, 5, D_IN], BF)
            nc.gpsimd.dma_start(out=brow_sb[:], in_=brows[None, :, :])
        else:
            brow_sb = None
        # later chunks: half-chunk DMAs (contiguous per partition)
        for ci in range(1, NCH):
            csz = chunks[ci]
            tbc = sing.tile([128, csz, QP], BF, name=f"tbc{ci}")
            nq = csz // 32
            for half in range(2 if nq > 2 else 1):
                lo = half * (nq // 2) * 32
                hi = csz if (nq <= 2 or half == 1) else (nq // 2) * 32
                nc.sync.dma_start(out=tbc[:, lo:hi, :],
                                  in_=bias_tr[:, ci * 128 + lo:ci * 128 + hi, :])
            for g in range(nq):
                tbs[(ci, g)] = tbc[:, g * 32:(g + 1) * 32, :]

        # ---------------- constants ----------------
        if need_ones_row:
            ones_row = sing.tile([1, max(kpad, D_IN)], BF)
            nc.vector.memset(ones_row[:], 1.0)
        ones_col = sing.tile([128, 1], BF)
        nc.vector.memset(ones_col[:], 1.0)
        eps_sb = sing.tile([128, 1], F32)
        nc.vector.memset(eps_sb[:], EPS)
        ident = sing.tile([128, 128], BF)
        masks.make_identity(nc, ident[:])

        # ---------------- LayerNorm(x) ----------------
        def ln_rows(dst_ap, xt):
            st6 = lnp.tile([128, 6], F32, tag="ln_st6")
            nc.vector.bn_stats(out=st6[:], in_=xt)
            mv = lnp.tile([128, 2], F32, tag="ln_mv")
            nc.vector.bn_aggr(out=mv[:], in_=st6[:])
            s = lnp.tile([128, 2], F32, tag="ln_s")
            nc.scalar.activation(s[:, 0:1], mv[:, 1:2], AF.Ln, bias=eps_sb[:, 0:1])
            nc.scalar.activation(s[:, 1:2], s[:, 0:1], AF.Exp, scale=-0.5)
            nc.vector.tensor_scalar(
                out=dst_ap, in0=xt, scalar1=mv[:, 0:1], scalar2=s[:, 1:2],
                op0=OP.subtract, op1=OP.mult,
            )

        xn_sb = sing.tile([128, nkr, D_IN], BF)
        xq_sb = sing.tile([128, D_IN], BF)

        def emit_ln():  # called after chunk-0 squares so DVE isn't blocked on xcomb
            for r in range(nkr):
                ln_rows(xn_sb[:, r, :], xall[:, r, :])
            ln_rows(xq_sb[:], xqt)

        # xnT/xqT via PE transposes (xbar dma_start_transpose occupies global
        # DMA ring slots and serializes ~6us behind LN on the scalar queue).
        # Emitted by emit_x_transposes AFTER chunk-0 projections in PE order.
        xnT = sing.tile([128, 2, kfull], BF)
        xqT = sing.tile([128, 2, QP], BF)

        def emit_x_transposes():
            slots = []
            for r in range(nkr):
                for h2 in range(2):
                    slots.append(xn_sb[:, r, h2 * 128:(h2 + 1) * 128])
            for h2 in range(2):
                slots.append(xq_sb[:, h2 * 128:(h2 + 1) * 128])
            for base in range(0, len(slots), 8):
                grp = slots[base:base + 8]
                xT_ps = ps_pt.tile([128, 8, 128], BF, tag="ptps", name="xT_ps")
                for i, src in enumerate(grp):
                    nc.tensor.transpose(xT_ps[:, i, :], src, ident[:])
                for i in range(len(grp)):
                    gi = base + i
                    if gi < nkr * 2:
                        r, h2 = divmod(gi, 2)
                        nc.vector.tensor_copy(xnT[:, h2, r * 128:(r + 1) * 128],
                                              xT_ps[:, i, :])
                    else:
                        nc.vector.tensor_copy(xqT[:, gi - nkr * 2, :], xT_ps[:, i, :])

        # squares: emitted per-chunk inside emit_chunk_front so a late chunk's
        # square never head-of-line-blocks the DVE queue behind its DMA.
        # Only the real query columns [0:qsq] are squared: the rest feed only
        # padded-query output rows, which stay partition-contained.
        sqs = {}

        def emit_squares(ci, csz):
            for g in range(csz // 32):
                tbg = tbs[(ci, g)]
                sqg = sqp.tile([128, 32, QP], BF, tag="sq", name=f"sq{ci}_{g}")
                eng = sq_engine(ci, g)
                nhh = 2 if (ci == 0 and g == 0) else 1  # finer for the 16-key first DMA
                for hh in range(nhh):
                    sl = slice(hh * (32 // nhh), (hh + 1) * (32 // nhh))
                    o, i0 = sqg[:, sl, 0:qsq], tbg[:, sl, 0:qsq]
                    if eng == "act":
                        nc.scalar.activation(o, i0, AF.Square)
                    else:
                        nc.vector.tensor_tensor(out=o, in0=i0, in1=i0, op=OP.mult)
                sqs[(ci, g)] = sqg

        P0 = {}

        def emit_phase0_mms():
            kT = sing.tile([128, 2, kpad], BF)
            for h2 in range(2):
                pk = ps_tile()[:, 0:kpad]
                nc.tensor.matmul(pk[:], lhsT=w_sb["k"][:, 0, h2 * 128:(h2 + 1) * 128],
                                 rhs=xnT[:, 0, 0:kpad], start=True, stop=(modes[1] != 2))
                nc.tensor.matmul(pk[:], lhsT=w_sb["k"][:, 1, h2 * 128:(h2 + 1) * 128],
                                 rhs=xnT[:, 1, 0:kpad], start=False, stop=(modes[1] != 2))
                if modes[1] == 2:
                    nc.tensor.matmul(pk[:], lhsT=brow_sb[:, 1, h2 * 128:(h2 + 1) * 128],
                                     rhs=ones_row[:, 0:kpad], start=False, stop=True)
                nc.scalar.copy(kT[:, h2, :], pk[:])
            qT = sing.tile([128, 2, QP], BF)
            for h2 in range(2):
                pq = ps_tile()[:, 0:QP]
                nc.tensor.matmul(pq[:], lhsT=w_sb["q"][:, 0, h2 * 128:(h2 + 1) * 128],
                                 rhs=xqT[:, 0, :], start=True, stop=(modes[0] != 2))
                nc.tensor.matmul(pq[:], lhsT=w_sb["q"][:, 1, h2 * 128:(h2 + 1) * 128],
                                 rhs=xqT[:, 1, :], start=False, stop=(modes[0] != 2))
                if modes[0] == 2:
                    nc.tensor.matmul(pq[:], lhsT=brow_sb[:, 0, h2 * 128:(h2 + 1) * 128],
                                     rhs=ones_row[:, 0:QP], start=False, stop=True)
                nc.scalar.copy(qT[:, h2, :], pq[:])

            v_ext = sing.tile([128, nkr, H, 33], BF)
            nc.vector.memset(v_ext[:], 1.0)
            for r in range(nkr):
                pv = ps_tile()[:, 0:D_IN]
                nc.tensor.matmul(pv[:], lhsT=xnT[:, 0, r * 128:(r + 1) * 128],
                                 rhs=w_sb["v"][:, 0, :], start=True, stop=(modes[2] != 2))
                nc.tensor.matmul(pv[:], lhsT=xnT[:, 1, r * 128:(r + 1) * 128],
                                 rhs=w_sb["v"][:, 1, :], start=False, stop=(modes[2] != 2))
                if modes[2] == 2:
                    nc.tensor.matmul(pv[:], lhsT=ones_row[:, 0:128],
                                     rhs=brow_sb[:, 2, :], start=False, stop=True)
                nc.vector.tensor_copy(v_ext[:, r, :, 0:32], pv[:].rearrange("p (h d) -> p h d", h=H))

            gate_sb = sing.tile([128, D_IN], F32)
            pg = ps_tile()[:, 0:D_IN]
            nc.tensor.matmul(pg[:], lhsT=xqT[:, 0, :], rhs=w_sb["g"][:, 0, :],
                             start=True, stop=(modes[3] != 2))
            nc.tensor.matmul(pg[:], lhsT=xqT[:, 1, :], rhs=w_sb["g"][:, 1, :],
                             start=False, stop=(modes[3] != 2))
            if modes[3] == 2:
                nc.tensor.matmul(pg[:], lhsT=ones_row[:, 0:128], rhs=brow_sb[:, 3, :],
                                 start=False, stop=True)
            # sigmoid(x+b) = 1/(1+exp(-x-b)); uniform b rides the activation bias
            if modes[3] == 1:
                nc.scalar.activation(gate_sb[:], pg[:], AF.Exp, scale=-1.0,
                                     bias=gb_sb[:, 0:1])
            else:
                nc.scalar.activation(gate_sb[:], pg[:], AF.Exp, scale=-1.0)
            nc.vector.tensor_scalar(out=gate_sb[:], in0=gate_sb[:], scalar1=1.0,
                                    scalar2=None, op0=OP.add)
            nc.vector.reciprocal_approx_fast(gate_sb[:], gate_sb[:])

            s_all = sing.tile([128, H, kpad], F32)
            for h in range(H):
                pS = ps_tile()[:, 0:kpad]
                base = 32 * (h % 4)
                nc.tensor.matmul(pS[:], lhsT=qT[base:base + 32, h // 4, :],
                                 rhs=kT[base:base + 32, h // 4, :],
                                 start=True, stop=True, tile_position=(base, 0))
                nc.vector.tensor_tensor(out=s_all[:, h, :], in0=pS[:], in1=mk_sb[:], op=OP.add)

            P0.update(kT=kT, qT=qT, v_ext=v_ext, gate_sb=gate_sb, s_all=s_all)

        # ---------------- per-chunk front: proj/ss matmuls + stats + t1 ----------------
        pvps = ps_pv.tile([128, H * 33], F32)

        def emit_chunk_front(ci, csz):
            if ci > 0:
                emit_squares(ci, csz)  # chunk 0's are emitted before LN
            t1 = scr.tile([128, 128 * P], F32, tag="fx1", name="t1")
            for g in range(csz // 32):
                tbg, sqg = tbs[(ci, g)], sqs[(ci, g)]
                rp = ps_tile()[:, 0:32 * P]
                for j in range(32):
                    nc.tensor.matmul(rp[:, j * P:j * P + 9], lhsT=tbg[:, j, :],
                                     rhs=wext_sb[:, 0:9], start=True, stop=True)
                for j in range(32):
                    nc.tensor.matmul(rp[:, j * P + 9:j * P + 10], lhsT=sqg[:, j, :],
                                     rhs=ones_col[:], start=True, stop=True)
                msq = scr.tile([128, 32], F32, tag="msq", name="msq")
                nc.scalar.activation(msq[:], _ap(rp, 8, [[P, 32]]), AF.Square)
                var_g = scr.tile([128, 32], F32, tag="var", name="var_g")
                nc.vector.scalar_tensor_tensor(out=var_g[:],
                                               in0=_ap(rp, 9, [[P, 32]]),
                                               scalar=1.0 / D_BIAS, in1=msq[:],
                                               op0=OP.mult, op1=OP.subtract)
                lnv = scr.tile([128, 32], F32, tag="lnv", name="lnv")
                nc.scalar.activation(lnv[:], var_g[:], AF.Ln, bias=eps_sb[:, 0:1])
                rinv_g = scr.tile([128, 32], F32, tag="rinv", name="rinv_g")
                nc.scalar.activation(rinv_g[:], lnv[:], AF.Exp, scale=-0.5)
                nc.vector.tensor_tensor(
                    out=t1[:, g * 32 * P:(g + 1) * 32 * P].rearrange("p (k c) -> p k c", c=P),
                    in0=rp.rearrange("p (k c) -> p k c", c=P),
                    in1=_ap(rinv_g[:], 0, [[1, 32], [0, P]]), op=OP.mult)
            return t1

        def emit_chunk_fixup(ci, csz, t1):
            """t2 = t1 + S (DVE, strided on head cols) -> p = exp (ACT)."""
            p_sb = scr.tile([128, 128 * P], BF, tag="p", name="p_sb")
            nseg = 2 if ci < NCH - 1 else 4
            kseg = max(csz // nseg, 8)
            nseg = csz // kseg
            for hf in range(nseg):
                o = hf * kseg * P
                nc.vector.tensor_tensor(
                    out=_ap(t1[:], o, [[P, kseg], [1, H]]),
                    in0=_ap(t1[:], o, [[P, kseg], [1, H]]),
                    in1=_ap(P0['s_all'][:], ci * 128 + hf * kseg, [[1, kseg], [kpad, H]]),
                    op=OP.add)
                nc.scalar.activation(p_sb[:, o:o + kseg * P], t1[:, o:o + kseg * P], AF.Exp)
            return p_sb

        def emit_chunk_back(ci, csz, p_sb):
            pT_ps = ps_pt.tile([128, H, 128], BF, tag="ptps", name="pT_ps")
            for h in range(H):
                nc.tensor.transpose(pT_ps[0:csz, h, :], _ap(p_sb[:], h, [[P, csz]]), ident[:])
            pT_sb = ptp.tile([128, H, 128], BF, tag="ptsb", name="pT_sb")
            nc.scalar.copy(pT_sb[0:csz, :, :], pT_ps[0:csz, :, :])
            for h in range(H):
                nc.tensor.matmul(pvps[:, h * 33:(h + 1) * 33], lhsT=pT_sb[0:csz, h, :],
                                 rhs=P0['v_ext'][0:csz, ci, h, :],
                                 start=(ci == 0 and h == 0), stop=(ci == NCH - 1))

        # ---------------- main schedule ----------------
        emit_squares(0, chunks[0])
        emit_ln()
        t1 = emit_chunk_front(0, chunks[0])
        emit_x_transposes()
        emit_phase0_mms()
        pending = (0, chunks[0], emit_chunk_fixup(0, chunks[0], t1))
        for ci in range(1, NCH):
            t1 = emit_chunk_front(ci, chunks[ci])
            emit_chunk_back(*pending)
            pending = (ci, chunks[ci], emit_chunk_fixup(ci, chunks[ci], t1))
        emit_chunk_back(*pending)

        # ---------------- phase 2: denominators, gate, output ----------------
        dn = sing.tile([128, 16], F32)
        nc.vector.tensor_scalar(out=dn[:, 0:8], in0=_ap(pvps[:], 32, [[33, 8]]),
                                scalar1=1e-30, scalar2=None, op0=OP.add)
        nc.vector.reciprocal_approx_fast(dn[:, 8:16], dn[:, 0:8])

        comb1 = sing.tile([128, D_IN], F32)
        nc.vector.tensor_tensor(out=comb1[:].rearrange("p (h d) -> p h d", h=H),
                                in0=_ap(pvps[:], 0, [[33, 8], [1, 32]]),
                                in1=P0['gate_sb'][:].rearrange("p (h d) -> p h d", h=H),
                                op=OP.mult)
        comb = sing.tile([128, D_IN], BF)
        nc.vector.tensor_tensor(out=comb[:].rearrange("p (h d) -> p h d", h=H),
                                in0=comb1[:].rearrange("p (h d) -> p h d", h=H),
                                in1=_ap(dn[:], 8, [[1, 8], [0, DH]]), op=OP.mult)

        cT_ps = ps_pt.tile([128, H, 128], BF, tag="ptps")
        for c in range(2):
            nc.tensor.transpose(cT_ps[:, c, :], comb[:, c * 128:(c + 1) * 128], ident[:])
        cT_sb = ptp.tile([128, 2, 128], BF, tag="ctsb")
        nc.vector.tensor_copy(cT_sb[:], cT_ps[:, 0:2, :])

        fin = ps_tile()[:, 0:D_IN]
        for c in range(2):
            nc.tensor.matmul(fin[:], lhsT=cT_sb[:, c, :], rhs=w_sb["o"][:, c, :],
                             start=(c == 0), stop=(modes[4] != 2 and c == 1))
        if modes[4] == 2:
            nc.tensor.matmul(fin[:], lhsT=ones_row[:, 0:128], rhs=brow_sb[:, 4, :],
                             start=False, stop=True)
        out_sb = sing.tile([128, D_IN], F32)
        nc.scalar.activation(out_sb[:], fin[:], AF.Copy, scale=rowm_sb[:, 0:1])
        nc.sync.dma_start(out=out[:, :], in_=out_sb[0:QP, :])

    # Steer insert_act_table_loads to the one set covering Square/Ln/Exp/Copy
    orig_tables = bacc.get_activation_tables
    keep = "natural_log_exp_and_others"

    def _patched(arch):
        t = orig_tables(arch)
        return {name: (fs if name == keep else set()) for name, fs in t.items()}

    bacc.get_activation_tables = _patched
    try:
        nc.compile()
    finally:
        bacc.get_activation_tables = orig_tables
    return nc


def _prep_common(inputs):
    ln_in_g = np.asarray(inputs["ln_in_g"], np.float64)
    ln_in_b = np.asarray(inputs["ln_in_b"], np.float64)
    ln_b_g = np.asarray(inputs["ln_b_g"], np.float64)
    Wq = np.asarray(inputs["Wq"], np.float64)
    Wk = np.asarray(inputs["Wk"], np.float64)
    Wv = np.asarray(inputs["Wv"], np.float64)
    Wg = np.asarray(inputs["Wg"], np.float64)
    Wb = np.asarray(inputs["Wb"], np.float64)
    Wo = np.asarray(inputs["Wo"], np.float64)
    bg = np.asarray(inputs["bg"], np.float64)
    bo = np.asarray(inputs["bo"], np.float64)

    def arr_w(w):  # [256, 256] -> [128, 2, 256] din-chunk grouping
        return np.ascontiguousarray(
            w.reshape(2, 128, D_IN).transpose(1, 0, 2)).astype(BF16)

    wall5 = np.stack([
        arr_w(Wq * ln_in_g[:, None]),
        arr_w(Wk * ln_in_g[:, None] * SCALE),
        arr_w(Wv * ln_in_g[:, None]),
        arr_w(Wg * ln_in_g[:, None]),
        arr_w(Wo),
    ], axis=1).reshape(128, 5 * 2 * D_IN)

    brow_rows = [
        ln_in_b @ Wq,
        (ln_in_b @ Wk) * SCALE,
        ln_in_b @ Wv,
        ln_in_b @ Wg + bg,
        bo,
    ]
    brows = np.stack(brow_rows).astype(BF16)

    modes = []
    for i, r in enumerate(brow_rows):
        if np.all(r == 0.0):
            modes.append(0)
        elif i == 3 and np.all(r == r[0]):
            modes.append(1)  # uniform gate bias -> activation bias operand
        else:
            modes.append(2)
    gbias_val = -float(brow_rows[3][0]) if modes[3] == 1 else 0.0

    c1 = ln_b_g @ Wb                        # [H]
    wext = np.zeros((D_BIAS, 16), np.float64)
    wext[:, 0:H] = Wb * ln_b_g[:, None] - c1[None, :] / D_BIAS
    wext[:, 8] = 1.0 / D_BIAS
    wext = wext.astype(BF16)
    wall = np.ascontiguousarray(np.concatenate([wall5, wext], axis=1))

    return dict(wall=wall, brows=brows), tuple(modes), gbias_val


def _plan(mask):
    """Compaction plan: per-batch unmasked key indices (uniform kpad) and
    per-core query index groups (queries pad to QP=128, always sufficient)."""
    kidx = [np.where(mask[b] != 0)[0] for b in range(B)]
    kmax = max(len(k) for k in kidx)
    kpad = max(32, -(-kmax // 32) * 32)
    qgroups = []
    for b in range(B):
        qgroups.extend(np.array_split(kidx[b], 4))
    return kidx, kpad, qgroups


def _make_in_maps(inputs):
    x = np.asarray(inputs["x"], np.float32)
    bias = np.asarray(inputs["bias"], np.float32)
    mask = np.asarray(inputs["mask"])
    common, modes, gbias_val = _prep_common(inputs)
    if not any(m == 2 for m in modes):
        common.pop("brows")  # dram param not declared when all bias rows elided
    kidx, kpad, qgroups = _plan(mask)
    nkr = (kpad + 127) // 128
    kfull = nkr * 128

    qsq = max(8, -(-max(len(q) for q in qgroups) // 8) * 8)
    in_maps = []
    for c in range(8):
        b = c // 4
        ks, qs = kidx[b], qgroups[c]
        K, Q = len(ks), len(qs)
        nat = bias[b][np.ix_(qs, ks)].astype(BF16)       # [Q, K, 128]
        bt = np.zeros((D_BIAS, kpad, QP), BF16)
        bt[:, :K, :Q] = nat.transpose(2, 1, 0)
        xk = np.zeros((kfull, D_IN), np.float32)
        xk[:K] = x[b, ks]
        # pre-arrange to [128, nkr*256]: partition p holds rows p, p+128, ...
        xk = xk.reshape(nkr, 128, D_IN).transpose(1, 0, 2).reshape(128, nkr * D_IN)
        xq = np.zeros((QP, D_IN), np.float32)
        xq[:Q] = x[b, qs]
        # xcomb = [x_q | x_k | mk | rowm | gbias | pad] in one contiguous DMA
        xc = np.zeros((128, (1 + nkr) * D_IN + kpad + 8), np.float32)
        xc[:, 0:D_IN] = xq
        xc[:, D_IN:(1 + nkr) * D_IN] = xk
        xc[:, (1 + nkr) * D_IN + K:(1 + nkr) * D_IN + kpad] = NEG  # mask pad keys
        xc[:Q, (1 + nkr) * D_IN + kpad] = 1.0                      # rowm
        xc[:, (1 + nkr) * D_IN + kpad + 1] = gbias_val
        in_maps.append(dict(
            bias_tr=np.ascontiguousarray(bt),
            xcomb=xc,
            **common,
        ))
    return in_maps, modes, kpad, qsq


def kernel(**inputs):
    mask = np.asarray(inputs["mask"])
    in_maps, modes, kpad, qsq = _make_in_maps(inputs)
    key = (kpad, qsq, modes)
    if key not in _CACHE:
        _CACHE[key] = _build(kpad, qsq, modes)
        _CACHE["last"] = _CACHE[key]
    nc = _CACHE[key]

    res = run_bass_kernel_spmd(nc, in_maps, list(range(8)))
    out = np.zeros((B, L, D_IN), np.float32)
    _, _, qgroups = _plan(mask)
    for c in range(8):
        b = c // 4
        qs = qgroups[c]
        out[b, qs] = res.results[c]["out"][:len(qs)]
    return out
